# revision 13
# baseline (speedup 1.0000x reference)
"""Trainium2 Bass kernel for the BiAttention problem (v3).

Math (per batch b, L=1024, D=256, sigma=4.0 a fixed logit shift):
  s0[i] = c[i,:] @ c_weight ;  s1[j] = c[j,:] @ q_weight
  S[i,j] = s0[i] + s1[j] + s2[i,j] + bias,  s2 = (c*cqw) @ q^T
  E1T[j,i] = exp(S^T - sigma): s2^T via bf16 GEMM, s1 via the ACT exp
      bias (per-partition j), s0 via an extra fp8-DoubleRow rank-1 pair
      (hi+lo residual) accumulated into the same PSUM group.
  S1 row-softmax over j: sigma cancels ->
      D1[i] = sum_j E1T[j,i] (ones-column),  C2Q = (E1T^T @ [q|1]) / D1
  Cross-batch softmax: ZT[j,i] = sum_b E1T_b  -> one bf16 [L,L] AllReduce
      computed IN [j,i] orientation so it pipelines with the exp stream.
  rZT = exp(-ln ZT) on ACT; rZ = DMA-XBAR transpose of rZT.
  W_b[j,d] = sum_i F_b[i,j] c[i,d],  F_b = E1f_b * rZ (bf16 GEMM);
      E1f_b = DMA-XBAR transpose of E1T_b (runs inside the AR window).
  Q2C = (E8T^T @ (W8hi+W8lo)) / D1q:  fp8 DoubleRow GEMM with W as an
      fp8 hi+lo residual pair and an fp8-consistent normalizer D1q from a
      tiny ones GEMM.  E8T = fp8 cast of E1T (gpsimd, in the AR window).
  out = concat(c, C2Q, c*C2Q, c*Q2C) on axis 0.
"""

import sys

import numpy as np

for _p in ("/opt/trn_rl_repo",):
    if _p not in sys.path:
        sys.path.insert(0, _p)

import ml_dtypes

import concourse.bacc as bacc
import concourse.mybir as mybir
import concourse.tile as tile
from concourse.bass_utils import run_bass_kernel_spmd

F32 = mybir.dt.float32
BF16 = mybir.dt.bfloat16
F8 = mybir.dt.float8e4
AF = mybir.ActivationFunctionType
ALU = mybir.AluOpType
DR = mybir.MatmulPerfMode.DoubleRow

NP_BF16 = ml_dtypes.bfloat16
NP_F8 = ml_dtypes.float8_e4m3

B, L, D = 16, 1024, 256
NCORES = 8
BPC = B // NCORES  # batches per core
P = 128
LB = L // P   # 8 row-blocks of 128
DB = D // P   # 2 K-tiles of 128
SIGMA = 4.0   # global logit shift (must be identical on every core)

_CACHE = {}


def _build_nc():
    nc = bacc.Bacc(
        "TRN2",
        target_bir_lowering=False,
        debug=False,
        num_devices=NCORES,
    )

    # ---- kernel I/O (packed per dtype to cut DMA trigger count) ----
    # qcq: [:, :, 0:1024] = q^T folded, [:, :, 1024:2048] = (c*cqw)^T folded
    qcq_d = nc.dram_tensor("qcq", [BPC, P, DB, 2 * L], BF16, kind="ExternalInput")
    s1c_d = nc.dram_tensor("s1c", [BPC, P, LB], F32, kind="ExternalInput")
    # s0 hi/lo residual pair, fp8, rank-1 rhs rows
    s0hl_d = nc.dram_tensor("s0hl", [BPC, 1, 2, L], F8, kind="ExternalInput")
    # qc: [:, :, :, 0:257] = [q | 1], [:, :, :, 257:513] = c
    qc_d = nc.dram_tensor("qc", [BPC, P, LB, 513], BF16, kind="ExternalInput")

    o_c2q = nc.dram_tensor("o_c2q", [BPC, L, D], BF16, kind="ExternalOutput")
    o_cc2q = nc.dram_tensor("o_cc2q", [BPC, L, D], BF16, kind="ExternalOutput")
    o_cq2c = nc.dram_tensor("o_cq2c", [BPC, L, D], BF16, kind="ExternalOutput")

    rg = [list(range(NCORES))]

    with tile.TileContext(nc) as tc:
        with (
            tc.tile_pool(name="dram", bufs=1, space="DRAM") as dram,
            tc.tile_pool(name="small", bufs=1) as small,
            tc.tile_pool(name="inp", bufs=1) as inp,
            tc.tile_pool(name="ET", bufs=2) as ETp,      # E1T, recycled as F16
            tc.tile_pool(name="Ef", bufs=2) as Efp,      # E1f [i,j]
            tc.tile_pool(name="F16", bufs=2) as F16p,    # F = E1f*rZ
            tc.tile_pool(name="Zc", bufs=2) as Zcp,      # zout chunks [j,i]
            tc.tile_pool(name="ln", bufs=2) as lnp,      # ln(Z) chunks
            tc.tile_pool(name="rZT", bufs=3) as rZTp,    # rZ^T per-jb tiles
            tc.tile_pool(name="rZ", bufs=1) as rZp,      # rZ [i,j] full
            tc.tile_pool(name="zl", bufs=2) as zlp,      # zin staging [j,i]
            tc.tile_pool(name="W16", bufs=2) as W16p,
            tc.tile_pool(name="st", bufs=2) as stp,      # batched out staging
            tc.tile_pool(name="sm2", bufs=4) as sm2p,    # small evac tiles
            tc.tile_pool(name="psA", bufs=2, space="PSUM") as psA,
            tc.tile_pool(name="psB", bufs=4, space="PSUM") as psB,
        ):
            # zin/zout live in [j,i] orientation: [128(j), jb, 1024(i)]
            zin = dram.tile([P, LB, L], BF16, name="zin")
            zout = dram.tile([P, LB, L], BF16, name="zout", addr_space="Shared")

            # ---- small per-partition vectors ----
            s1c = [small.tile([P, LB], F32, name=f"s1c{b}") for b in range(BPC)]
            rD1 = [small.tile([P, LB], F32, name=f"rD1{b}") for b in range(BPC)]
            onesp = small.tile([1, 2, P], F8, name="onesp")
            nc.gpsimd.memset(onesp[:], 1.0)
            s0hl = [small.tile([1, 2, L], F8, name=f"s0hl{b}") for b in range(BPC)]
            for b in range(BPC):
                nc.sync.dma_start(s1c[b][:], s1c_d[b])
                nc.sync.dma_start(s0hl[b][:], s0hl_d[b])

            # ---- bulk input loads ----
            qcq = []
            for b in range(BPC):
                t = inp.tile([P, DB, 2 * L], BF16, name=f"qcq_{b}")
                nc.sync.dma_start(t[:], qcq_d[b])
                qcq.append(t)
            qc = []
            for b in range(BPC):
                t = inp.tile([P, LB, 513], BF16, name=f"qc_{b}")
                nc.sync.dma_start(t[:], qc_d[b])
                qc.append(t)

            # ---- phase 1 (jb-major): E1T = exp(S^T - sigma); zl; zin ----
            E1T = []
            for b in range(BPC):
                E1T.append(ETp.tile([P, LB, L], BF16, name=f"E1T{b}", tag="ET"))
            for jb in range(LB):
                for b in range(BPC):
                    pv = psA.tile([P, L], F32, name="pv", tag="pv")
                    for n in range(2):
                        sl = slice(n * 512, (n + 1) * 512)
                        for h in range(DB):
                            nc.tensor.matmul(
                                pv[:, sl],
                                qcq[b][:, h, jb * P:(jb + 1) * P],
                                qcq[b][:, h, L + n * 512:L + (n + 1) * 512],
                                start=(h == 0), stop=False,
                            )
                        # s0[i] via fp8 DoubleRow hi/lo rank-1 pair
                        nc.tensor.matmul(
                            pv[:, sl],
                            onesp[:],
                            s0hl[b][:, :, sl],
                            start=False, stop=True, perf_mode=DR,
                        )
                    nc.scalar.activation(
                        E1T[b][:, jb, :], pv[:], AF.Exp,
                        bias=s1c[b][:, jb:jb + 1],
                    )
                # zl[jb] = E1T0[jb] + E1T1[jb]  (cross-batch partial sum)
                zl = zlp.tile([P, L], BF16, name=f"zl{jb}", tag="zl")
                nc.vector.tensor_add(zl[:], E1T[0][:, jb, :], E1T[1][:, jb, :])
                nc.sync.dma_start(zin[:, jb, :], zl[:])

            # ---- cross-batch softmax denominator AllReduce ----
            nc.gpsimd.collective_compute(
                "AllReduce", ALU.add, replica_groups=rg,
                ins=[zin.opt()], outs=[zout.opt()],
            )

            # ==== AR window work (independent of Z) ====
            # E1f = transpose(E1T) via DMA XBAR (ACT-triggered)
            E1f = []
            for b in range(BPC):
                E1f.append(Efp.tile([P, LB, L], BF16, name=f"E1f{b}", tag="Ef"))
            for b in range(BPC):
                for jb in range(LB):
                    nc.scalar.dma_start(
                        E1f[b][:, :, jb * P:(jb + 1) * P],
                        E1T[b][:, jb, :],
                        transpose=True,
                    )
            # C2Q = (E1T^T @ [q|1]) / D1  (bf16 GEMM), batched stores
            for b in range(BPC):
                st1 = stp.tile([P, LB, D], BF16, name=f"stc2q{b}", tag="st1")
                st2 = stp.tile([P, LB, D], BF16, name=f"stcc{b}", tag="st2")
                for ib in range(LB):
                    ps = psB.tile([P, 512], F32, name="psc", tag="ps")
                    for jk in range(LB):
                        nc.tensor.matmul(
                            ps[:, 0:D + 1],
                            E1T[b][:, jk, ib * P:(ib + 1) * P],
                            qc[b][:, jk, 0:D + 1],
                            start=(jk == 0), stop=(jk == LB - 1),
                        )
                    nc.vector.reciprocal_approx_fast(
                        out=rD1[b][:, ib:ib + 1], in_=ps[:, D:D + 1],
                    )
                    nc.scalar.activation(
                        st1[:, ib, :], ps[:, 0:D], AF.Copy, bias=0.0,
                        scale=rD1[b][:, ib:ib + 1],
                    )
                    nc.vector.tensor_mul(
                        st2[:, ib, :], st1[:, ib, :],
                        qc[b][:, ib, 257:513],
                    )
                nc.sync.dma_start(
                    o_c2q[b].rearrange("(m p) d -> p m d", p=P), st1[:])
                nc.sync.dma_start(
                    o_cc2q[b].rearrange("(m p) d -> p m d", p=P), st2[:])

            # ==== post-AR: rZT = exp(-ln ZT) on ACT; transpose to rZ ====
            rZ = rZp.tile([P, LB, L], BF16, name="rZ", tag="rZ")
            for ch in range(LB):
                zc = Zcp.tile([P, L], BF16, name=f"Zc{ch}", tag="Zc")
                nc.scalar.dma_start(zc[:], zout[:, ch, :])
                lz = lnp.tile([P, L], F32, name=f"ln{ch}", tag="ln")
                nc.scalar.activation(lz[:], zc[:], AF.Ln)
                rzt = rZTp.tile([P, L], BF16, name=f"rZT{ch}", tag="rZT")
                nc.scalar.activation(rzt[:], lz[:], AF.Exp, scale=-1.0)
                nc.scalar.dma_start(
                    rZ[:, :, ch * P:(ch + 1) * P],
                    rzt[:],
                    transpose=True,
                )

            # ---- F = E1f * rZ (bf16 DVE 2x); W bf16; Q2C fp8-DR ----
            for b in range(BPC):
                f16 = F16p.tile([P, LB, L], BF16, name=f"F16_{b}", tag="F16")
                for ib in range(LB):
                    nc.vector.tensor_mul(
                        f16[:, ib, :], E1f[b][:, ib, :], rZ[:, ib, :]
                    )
                # W = F^T @ c (bf16) -> W16 bf16
                w16 = W16p.tile([P, LB, D], BF16, name=f"W16_{b}", tag="W16")
                for jb in range(LB):
                    ps = psB.tile([P, 512], F32, name="psw", tag="ps")
                    for ik in range(LB):
                        nc.tensor.matmul(
                            ps[:, 0:D],
                            f16[:, ik, jb * P:(jb + 1) * P],
                            qc[b][:, ik, 257:513],
                            start=(ik == 0), stop=(ik == LB - 1),
                        )
                    if jb % 2 == 0:
                        nc.scalar.activation(w16[:, jb, :], ps[:, 0:D], AF.Copy)
                    else:
                        nc.vector.tensor_copy(out=w16[:, jb, :], in_=ps[:, 0:D])

                # Q2C = (E1T^T @ W16) / D1  (bf16, reuses rD1)
                st3 = stp.tile([P, LB, D], BF16, name=f"stq2c{b}", tag="st1")
                for ib in range(LB):
                    psq = psB.tile([P, 512], F32, name="psq", tag="ps")
                    for jk in range(LB):
                        nc.tensor.matmul(
                            psq[:, 0:D],
                            E1T[b][:, jk, ib * P:(ib + 1) * P],
                            w16[:, jk, :],
                            start=(jk == 0), stop=(jk == LB - 1),
                        )
                    q2ct = sm2p.tile([P, D], BF16, name="q2ct", tag="q2c")
                    nc.scalar.activation(
                        q2ct[:], psq[:, 0:D], AF.Copy, bias=0.0,
                        scale=rD1[b][:, ib:ib + 1],
                    )
                    nc.vector.tensor_mul(
                        st3[:, ib, :], q2ct[:], qc[b][:, ib, 257:513],
                    )
                nc.sync.dma_start(
                    o_cq2c[b].rearrange("(m p) d -> p m d", p=P), st3[:])

    nc.compile()
    return nc


def _get_nc():
    if "nc" not in _CACHE:
        _CACHE["nc"] = _build_nc()
    return _CACHE["nc"]


def kernel(c, q, c_mask=None, q_mask=None, c_weight=None, q_weight=None,
           cq_weight=None, bias=None, _trace=False, **_ignored):
    c = np.ascontiguousarray(np.asarray(c, dtype=np.float32))
    q = np.ascontiguousarray(np.asarray(q, dtype=np.float32))
    c_weight = np.asarray(c_weight, dtype=np.float32).reshape(D, 1)
    q_weight = np.asarray(q_weight, dtype=np.float32).reshape(D, 1)
    cq_weight = np.asarray(cq_weight, dtype=np.float32).reshape(D)
    bias_v = float(np.asarray(bias, dtype=np.float32).reshape(-1)[0])

    # Host-side prep is O(B*L*D): tiny GEMVs, layout shuffles, dtype casts.
    s0 = (c @ c_weight)[:, :, 0]            # [B, L]
    s1 = (c @ q_weight)[:, :, 0]            # [B, L]
    cq = c * cq_weight[None, None, :]       # [B, L, D]

    # [j,i]-side operands, K=d folded as (h,p): packed [q^T | cq^T]
    qT = q.reshape(B, L, DB, P).transpose(0, 3, 2, 1)
    cqT = cq.reshape(B, L, DB, P).transpose(0, 3, 2, 1)
    qcq = np.ascontiguousarray(
        np.concatenate([qT, cqT], axis=3)).astype(NP_BF16)
    # s1 column layout [128, LB] with the bias/sigma shift
    s1c = np.ascontiguousarray(
        (s1 + bias_v - SIGMA).reshape(B, LB, P).transpose(0, 2, 1))
    # s0 as fp8 hi/lo rank-1 rows [1, 2, L]
    s0hi = s0.astype(NP_F8)
    s0lo = (s0 - s0hi.astype(np.float32)).astype(NP_F8)
    s0hl = np.ascontiguousarray(
        np.stack([s0hi, s0lo], axis=1)[:, None, :, :]).reshape(B, 1, 2, L)
    # natural layouts packed: [q | 1 | c]  -> [128, LB, 513]
    qn = q.reshape(B, LB, P, D).transpose(0, 2, 1, 3)
    cn = c.reshape(B, LB, P, D).transpose(0, 2, 1, 3)
    qc = np.ascontiguousarray(np.concatenate(
        [qn, np.ones((B, P, LB, 1), np.float32), cn], axis=3)).astype(NP_BF16)

    nc = _get_nc()
    in_maps = []
    for kk in range(NCORES):
        sl = slice(kk * BPC, (kk + 1) * BPC)
        in_maps.append({
            "qcq": qcq[sl],
            "s1c": np.ascontiguousarray(s1c[sl]),
            "s0hl": np.ascontiguousarray(s0hl[sl]),
            "qc": qc[sl],
        })

    res = run_bass_kernel_spmd(
        nc, in_maps, core_ids=list(range(NCORES)), trace=_trace
    )
    _CACHE["last_result"] = res

    out = np.empty((4 * B, L, D), dtype=np.float32)
    out[0:B] = c

    def unshuffle(r):
        # [P, LB, D] -> [L, D]
        return np.asarray(r, dtype=np.float32).transpose(1, 0, 2).reshape(L, D)

    for kk in range(NCORES):
        sl = slice(kk * BPC, (kk + 1) * BPC)
        r = res.results[kk]
        for b in range(BPC):
            g = kk * BPC + b
            out[B + g] = np.asarray(r["o_c2q"][b], dtype=np.float32)
            out[2 * B + g] = np.asarray(r["o_cc2q"][b], dtype=np.float32)
            out[3 * B + g] = np.asarray(r["o_cq2c"][b], dtype=np.float32)
    return out


# revision 16
# speedup vs baseline: 1.0784x; 1.0784x over previous
"""Trainium2 Bass kernel for the BiAttention problem (v3).

Math (per batch b, L=1024, D=256, sigma=4.0 a fixed logit shift):
  s0[i] = c[i,:] @ c_weight ;  s1[j] = c[j,:] @ q_weight
  S[i,j] = s0[i] + s1[j] + s2[i,j] + bias,  s2 = (c*cqw) @ q^T
  E1T[j,i] = exp(S^T - sigma): s2^T via bf16 GEMM, s1 via the ACT exp
      bias (per-partition j), s0 via an extra fp8-DoubleRow rank-1 pair
      (hi+lo residual) accumulated into the same PSUM group.
  S1 row-softmax over j: sigma cancels ->
      D1[i] = sum_j E1T[j,i] (ones-column),  C2Q = (E1T^T @ [q|1]) / D1
  Cross-batch softmax: ZT[j,i] = sum_b E1T_b  -> one bf16 [L,L] AllReduce
      computed IN [j,i] orientation so it pipelines with the exp stream.
  rZT = exp(-ln ZT) on ACT; rZ = DMA-XBAR transpose of rZT.
  W_b[j,d] = sum_i F_b[i,j] c[i,d],  F_b = E1f_b * rZ (bf16 GEMM);
      E1f_b = DMA-XBAR transpose of E1T_b (runs inside the AR window).
  Q2C = (E8T^T @ (W8hi+W8lo)) / D1q:  fp8 DoubleRow GEMM with W as an
      fp8 hi+lo residual pair and an fp8-consistent normalizer D1q from a
      tiny ones GEMM.  E8T = fp8 cast of E1T (gpsimd, in the AR window).
  out = concat(c, C2Q, c*C2Q, c*Q2C) on axis 0.
"""

import sys

import numpy as np

for _p in ("/opt/trn_rl_repo",):
    if _p not in sys.path:
        sys.path.insert(0, _p)

import ml_dtypes

import concourse.bacc as bacc
import concourse.mybir as mybir
import concourse.tile as tile
from concourse.bass_utils import run_bass_kernel_spmd

F32 = mybir.dt.float32
BF16 = mybir.dt.bfloat16
F8 = mybir.dt.float8e4
AF = mybir.ActivationFunctionType
ALU = mybir.AluOpType
DR = mybir.MatmulPerfMode.DoubleRow

NP_BF16 = ml_dtypes.bfloat16
NP_F8 = ml_dtypes.float8_e4m3

B, L, D = 16, 1024, 256
NCORES = 8
BPC = B // NCORES  # batches per core
P = 128
LB = L // P   # 8 row-blocks of 128
DB = D // P   # 2 K-tiles of 128
SIGMA = 4.0   # global logit shift (must be identical on every core)

_CACHE = {}


def _build_nc():
    nc = bacc.Bacc(
        "TRN2",
        target_bir_lowering=False,
        debug=False,
        num_devices=NCORES,
    )

    # ---- kernel I/O (packed per dtype to cut DMA trigger count) ----
    # qcq: [:, :, 0:1024] = q^T folded, [:, :, 1024:2048] = (c*cqw)^T folded
    qcq_d = nc.dram_tensor("qcq", [BPC, P, DB, 2 * L], BF16, kind="ExternalInput")
    s1c_d = nc.dram_tensor("s1c", [BPC, P, LB], F32, kind="ExternalInput")
    # s0 hi/lo residual pair, fp8, rank-1 rhs rows
    s0hl_d = nc.dram_tensor("s0hl", [BPC, 1, 2, L], F8, kind="ExternalInput")
    # qc: [:, :, :, 0:257] = [q | 1], [:, :, :, 257:513] = c
    qc_d = nc.dram_tensor("qc", [BPC, P, LB, 513], BF16, kind="ExternalInput")

    o_c2q = nc.dram_tensor("o_c2q", [BPC, L, D], BF16, kind="ExternalOutput")
    o_cc2q = nc.dram_tensor("o_cc2q", [BPC, L, D], BF16, kind="ExternalOutput")
    o_cq2c = nc.dram_tensor("o_cq2c", [BPC, L, D], BF16, kind="ExternalOutput")

    rg = [list(range(NCORES))]

    with tile.TileContext(nc) as tc:
        with (
            tc.tile_pool(name="dram", bufs=1, space="DRAM") as dram,
            tc.tile_pool(name="small", bufs=1) as small,
            tc.tile_pool(name="inp", bufs=1) as inp,
            tc.tile_pool(name="ET", bufs=2) as ETp,      # E1T, recycled as F16
            tc.tile_pool(name="Ef", bufs=2) as Efp,      # E1f [i,j]
            tc.tile_pool(name="F16", bufs=2) as F16p,    # F = E1f*rZ
            tc.tile_pool(name="Zc", bufs=2) as Zcp,      # zout chunks [j,i]
            tc.tile_pool(name="ln", bufs=2) as lnp,      # ln(Z) chunks
            tc.tile_pool(name="rZT", bufs=3) as rZTp,    # rZ^T per-jb tiles
            tc.tile_pool(name="rZ", bufs=1) as rZp,      # rZ [i,j] full
            tc.tile_pool(name="zl", bufs=2) as zlp,      # zin staging [j,i]
            tc.tile_pool(name="W16", bufs=2) as W16p,
            tc.tile_pool(name="st", bufs=2) as stp,      # batched out staging
            tc.tile_pool(name="sm2", bufs=4) as sm2p,    # small evac tiles
            tc.tile_pool(name="psA", bufs=5, space="PSUM") as psA,
            tc.tile_pool(name="psB", bufs=3, space="PSUM") as psB,
        ):
            # zin/zout live in [j,i] orientation: [128(j), jb, 1024(i)]
            zin = dram.tile([P, LB, L], BF16, name="zin")
            zout = dram.tile([P, LB, L], BF16, name="zout", addr_space="Shared")

            # ---- small per-partition vectors ----
            s1c = [small.tile([P, LB], F32, name=f"s1c{b}") for b in range(BPC)]
            rD1 = [small.tile([P, LB], F32, name=f"rD1{b}") for b in range(BPC)]
            onesp = small.tile([1, 2, P], F8, name="onesp")
            nc.gpsimd.memset(onesp[:], 1.0)
            s0hl = [small.tile([1, 2, L], F8, name=f"s0hl{b}") for b in range(BPC)]
            for b in range(BPC):
                nc.sync.dma_start(s1c[b][:], s1c_d[b])
                nc.sync.dma_start(s0hl[b][:], s0hl_d[b])

            # ---- bulk input loads ----
            qcq = []
            for b in range(BPC):
                t = inp.tile([P, DB, 2 * L], BF16, name=f"qcq_{b}")
                nc.sync.dma_start(t[:], qcq_d[b])
                qcq.append(t)
            qc = []
            for b in range(BPC):
                t = inp.tile([P, LB, 513], BF16, name=f"qc_{b}")
                nc.sync.dma_start(t[:], qc_d[b])
                qc.append(t)

            # ---- phase 1 (jb-major): E1T = exp(S^T - sigma); zl; zin ----
            E1T = []
            for b in range(BPC):
                E1T.append(ETp.tile([P, LB, L], BF16, name=f"E1T{b}", tag="ET"))
            for jb in range(LB):
                for b in range(BPC):
                    for n in range(2):
                        sl = slice(n * 512, (n + 1) * 512)
                        pv = psA.tile([P, 512], F32, name="pv", tag="pv")
                        for h in range(DB):
                            nc.tensor.matmul(
                                pv[:],
                                qcq[b][:, h, jb * P:(jb + 1) * P],
                                qcq[b][:, h, L + n * 512:L + (n + 1) * 512],
                                start=(h == 0), stop=False,
                            )
                        # s0[i] via fp8 DoubleRow hi/lo rank-1 pair
                        nc.tensor.matmul(
                            pv[:],
                            onesp[:],
                            s0hl[b][:, :, sl],
                            start=False, stop=True, perf_mode=DR,
                        )
                        nc.scalar.activation(
                            E1T[b][:, jb, sl], pv[:], AF.Exp,
                            bias=s1c[b][:, jb:jb + 1],
                        )
                # zl[jb] = E1T0[jb] + E1T1[jb]  (cross-batch partial sum)
                zl = zlp.tile([P, L], BF16, name=f"zl{jb}", tag="zl")
                nc.vector.tensor_add(zl[:], E1T[0][:, jb, :], E1T[1][:, jb, :])
                nc.sync.dma_start(zin[:, jb, :], zl[:])

            # ---- cross-batch softmax denominator AllReduce ----
            nc.gpsimd.collective_compute(
                "AllReduce", ALU.add, replica_groups=rg,
                ins=[zin.opt()], outs=[zout.opt()],
            )

            # ==== AR window work (independent of Z) ====
            # E1f = transpose(E1T) via DMA XBAR (ACT-triggered)
            E1f = []
            for b in range(BPC):
                E1f.append(Efp.tile([P, LB, L], BF16, name=f"E1f{b}", tag="Ef"))
            for b in range(BPC):
                for jb in range(LB):
                    nc.sync.dma_start(
                        E1f[b][:, :, jb * P:(jb + 1) * P],
                        E1T[b][:, jb, :],
                        transpose=True,
                    )
            # C2Q = (E1T^T @ [q|1]) / D1  (bf16 GEMM), batched stores
            for b in range(BPC):
                st1 = stp.tile([P, LB, D], BF16, name=f"stc2q{b}", tag="st1")
                st2 = stp.tile([P, LB, D], BF16, name=f"stcc{b}", tag="st2")
                for ib in range(LB):
                    ps = psB.tile([P, 512], F32, name="psc", tag="ps")
                    for jk in range(LB):
                        nc.tensor.matmul(
                            ps[:, 0:D + 1],
                            E1T[b][:, jk, ib * P:(ib + 1) * P],
                            qc[b][:, jk, 0:D + 1],
                            start=(jk == 0), stop=(jk == LB - 1),
                        )
                    nc.vector.reciprocal_approx_fast(
                        out=rD1[b][:, ib:ib + 1], in_=ps[:, D:D + 1],
                    )
                    nc.scalar.activation(
                        st1[:, ib, :], ps[:, 0:D], AF.Copy, bias=0.0,
                        scale=rD1[b][:, ib:ib + 1],
                    )
                    nc.vector.tensor_mul(
                        st2[:, ib, :], st1[:, ib, :],
                        qc[b][:, ib, 257:513],
                    )
                nc.sync.dma_start(
                    o_c2q[b].rearrange("(m p) d -> p m d", p=P), st1[:])
                nc.sync.dma_start(
                    o_cc2q[b].rearrange("(m p) d -> p m d", p=P), st2[:])

            # ==== post-AR: rZT = exp(-ln ZT) on ACT; transpose to rZ ====
            rZ = rZp.tile([P, LB, L], BF16, name="rZ", tag="rZ")
            # chunks 0..3: f32 recip on DVE (gpsimd cast-load + cast-store)
            for ch in range(4):
                zf = Zcp.tile([P, L], F32, name=f"Zf{ch}", tag="Zc")
                nc.gpsimd.dma_start(zf[:], zout[:, ch, :])
                r32 = lnp.tile([P, L], F32, name=f"r32_{ch}", tag="ln")
                nc.vector.reciprocal_approx_fast(out=r32[:], in_=zf[:])
                rzt = rZTp.tile([P, L], BF16, name=f"rZTa{ch}", tag="rZT")
                nc.gpsimd.dma_start(rzt[:], r32[:])
                nc.sync.dma_start(
                    rZ[:, :, ch * P:(ch + 1) * P],
                    rzt[:],
                    transpose=True,
                )
            # chunks 4..7: exp(-ln Z) on ACT from bf16 loads
            for ch in range(4, LB):
                zc = Zcp.tile([P, L], F32, name=f"Zc{ch}", tag="Zc")
                zc16 = zc.bitcast(BF16)
                nc.scalar.dma_start(zc16[:, 0:L], zout[:, ch, :])
                lz = lnp.tile([P, L], F32, name=f"ln{ch}", tag="ln")
                nc.scalar.activation(lz[:], zc16[:, 0:L], AF.Ln)
                rzt = rZTp.tile([P, L], BF16, name=f"rZTb{ch}", tag="rZT")
                nc.scalar.activation(rzt[:], lz[:], AF.Exp, scale=-1.0)
                nc.sync.dma_start(
                    rZ[:, :, ch * P:(ch + 1) * P],
                    rzt[:],
                    transpose=True,
                )

            # ---- F = E1f * rZ (bf16 DVE 2x); W bf16; Q2C fp8-DR ----
            for b in range(BPC):
                f16 = F16p.tile([P, LB, L], BF16, name=f"F16_{b}", tag="F16")
                for ib in range(LB):
                    nc.vector.tensor_mul(
                        f16[:, ib, :], E1f[b][:, ib, :], rZ[:, ib, :]
                    )
                # W = F^T @ c (bf16) -> W16 bf16
                w16 = W16p.tile([P, LB, D], BF16, name=f"W16_{b}", tag="W16")
                for jb in range(LB):
                    ps = psB.tile([P, 512], F32, name="psw", tag="ps")
                    for ik in range(LB):
                        nc.tensor.matmul(
                            ps[:, 0:D],
                            f16[:, ik, jb * P:(jb + 1) * P],
                            qc[b][:, ik, 257:513],
                            start=(ik == 0), stop=(ik == LB - 1),
                        )
                    if jb % 2 == 0:
                        nc.scalar.activation(w16[:, jb, :], ps[:, 0:D], AF.Copy)
                    else:
                        nc.vector.tensor_copy(out=w16[:, jb, :], in_=ps[:, 0:D])

                # Q2C = (E1T^T @ W16) / D1  (bf16, reuses rD1)
                st3 = stp.tile([P, LB, D], BF16, name=f"stq2c{b}", tag="st1")
                for ib in range(LB):
                    psq = psB.tile([P, 512], F32, name="psq", tag="ps")
                    for jk in range(LB):
                        nc.tensor.matmul(
                            psq[:, 0:D],
                            E1T[b][:, jk, ib * P:(ib + 1) * P],
                            w16[:, jk, :],
                            start=(jk == 0), stop=(jk == LB - 1),
                        )
                    q2ct = sm2p.tile([P, D], BF16, name="q2ct", tag="q2c")
                    nc.scalar.activation(
                        q2ct[:], psq[:, 0:D], AF.Copy, bias=0.0,
                        scale=rD1[b][:, ib:ib + 1],
                    )
                    nc.gpsimd.tensor_mul(
                        st3[:, ib, :], q2ct[:], qc[b][:, ib, 257:513],
                    )
                nc.sync.dma_start(
                    o_cq2c[b].rearrange("(m p) d -> p m d", p=P), st3[:])

    nc.compile()
    return nc


def _get_nc():
    if "nc" not in _CACHE:
        _CACHE["nc"] = _build_nc()
    return _CACHE["nc"]


def kernel(c, q, c_mask=None, q_mask=None, c_weight=None, q_weight=None,
           cq_weight=None, bias=None, _trace=False, **_ignored):
    c = np.ascontiguousarray(np.asarray(c, dtype=np.float32))
    q = np.ascontiguousarray(np.asarray(q, dtype=np.float32))
    c_weight = np.asarray(c_weight, dtype=np.float32).reshape(D, 1)
    q_weight = np.asarray(q_weight, dtype=np.float32).reshape(D, 1)
    cq_weight = np.asarray(cq_weight, dtype=np.float32).reshape(D)
    bias_v = float(np.asarray(bias, dtype=np.float32).reshape(-1)[0])

    # Host-side prep is O(B*L*D): tiny GEMVs, layout shuffles, dtype casts.
    s0 = (c @ c_weight)[:, :, 0]            # [B, L]
    s1 = (c @ q_weight)[:, :, 0]            # [B, L]
    cq = c * cq_weight[None, None, :]       # [B, L, D]

    # [j,i]-side operands, K=d folded as (h,p): packed [q^T | cq^T]
    qT = q.reshape(B, L, DB, P).transpose(0, 3, 2, 1)
    cqT = cq.reshape(B, L, DB, P).transpose(0, 3, 2, 1)
    qcq = np.ascontiguousarray(
        np.concatenate([qT, cqT], axis=3)).astype(NP_BF16)
    # s1 column layout [128, LB] with the bias/sigma shift
    s1c = np.ascontiguousarray(
        (s1 + bias_v - SIGMA).reshape(B, LB, P).transpose(0, 2, 1))
    # s0 as fp8 hi/lo rank-1 rows [1, 2, L]
    s0hi = s0.astype(NP_F8)
    s0lo = (s0 - s0hi.astype(np.float32)).astype(NP_F8)
    s0hl = np.ascontiguousarray(
        np.stack([s0hi, s0lo], axis=1)[:, None, :, :]).reshape(B, 1, 2, L)
    # natural layouts packed: [q | 1 | c]  -> [128, LB, 513]
    qn = q.reshape(B, LB, P, D).transpose(0, 2, 1, 3)
    cn = c.reshape(B, LB, P, D).transpose(0, 2, 1, 3)
    qc = np.ascontiguousarray(np.concatenate(
        [qn, np.ones((B, P, LB, 1), np.float32), cn], axis=3)).astype(NP_BF16)

    nc = _get_nc()
    in_maps = []
    for kk in range(NCORES):
        sl = slice(kk * BPC, (kk + 1) * BPC)
        in_maps.append({
            "qcq": qcq[sl],
            "s1c": np.ascontiguousarray(s1c[sl]),
            "s0hl": np.ascontiguousarray(s0hl[sl]),
            "qc": qc[sl],
        })

    res = run_bass_kernel_spmd(
        nc, in_maps, core_ids=list(range(NCORES)), trace=_trace
    )
    _CACHE["last_result"] = res

    out = np.empty((4 * B, L, D), dtype=np.float32)
    out[0:B] = c

    def unshuffle(r):
        # [P, LB, D] -> [L, D]
        return np.asarray(r, dtype=np.float32).transpose(1, 0, 2).reshape(L, D)

    for kk in range(NCORES):
        sl = slice(kk * BPC, (kk + 1) * BPC)
        r = res.results[kk]
        for b in range(BPC):
            g = kk * BPC + b
            out[B + g] = np.asarray(r["o_c2q"][b], dtype=np.float32)
            out[2 * B + g] = np.asarray(r["o_cc2q"][b], dtype=np.float32)
            out[3 * B + g] = np.asarray(r["o_cq2c"][b], dtype=np.float32)
    return out
